# revision 48
# baseline (speedup 1.0000x reference)
"""MDTA Bass kernel for 8 TRN2 NeuronCores, two SPMD launches (optimized).

Reference semantics (row-major reshape!): q.reshape(B,HEADS,HW,D) maps
  tensor[b,hd,s,d] = conv[b, 16*hd+ci, y, 16*xs+d]  with s = ci*1024+y*8+xs
so the attention feature axis d is the LOW 4 BITS OF x, and
  attn[hd,d,j] = sum_{ci,y,xs} k2[16hd+ci,y,16xs+d]*q2[16hd+ci,y,16xs+j]
  out_conv[b,16hd+j, ci*8+y//16, (y%16)*8+xs] = sum_d v2[16hd+ci,y,16xs+d]*P[d,j]

Launch 1 (spatial shards: b x quarter-of-H, 1-row halo): LayerNorm via
bn_stats, the three conv1x1+conv3x3 chains (q,k in fp32r; v fully bf16),
and per-head 128x128 "pair" matrices via two levels of PE transposes.
Host: extracts strip-diagonals -> gram, softmax -> P, builds block-diag
PSTACK and per-output-row VROW (bf16) rearrangements of v. Launch 2:
P-apply + wo-conv1x1 (bf16 matmuls) + fp32 residual.

PSUM->SBUF copies alternate DVE/ACT; the bf16 cast of xn runs on gpsimd.
"""

import os
from contextlib import ExitStack

import numpy as np
import ml_dtypes

import concourse.bacc as bacc
import concourse.bass as bass
import concourse.mybir as mybir
import concourse.tile as tile
from concourse import bass_utils

F32 = mybir.dt.float32
F32R = mybir.dt.float32r
BF16 = mybir.dt.bfloat16
AX = mybir.AxisListType
ALU = mybir.AluOpType
ACT = mybir.ActivationFunctionType

B, C, H, W = 2, 128, 128, 128
HEADS, D = 8, 16
EPS = 1e-5
RPC = H // 4
RH = RPC + 2
NPIX = RPC * W          # 4096
NHAL = RH * W           # 4352
WP = W + 2

NBF = ml_dtypes.bfloat16

_CACHE = {}


class _Alt:
    """Alternate copy engines: mostly DVE, every `act_every`-th op on ACT."""

    def __init__(self, nc, act_every=None):
        self.nc = nc
        self.i = 0
        if act_every is None:
            act_every = int(os.environ.get("KALT", "3"))
        self.act_every = act_every

    def copy(self, dst, src):
        self.i += 1
        if self.i % self.act_every == 0:
            self.nc.scalar.copy(dst, src)
        else:
            self.nc.vector.tensor_copy(dst, src)


def _round_f32r(nc, pool, name, src_ap, shape, stage_pool=None, eng=None):
    """DMA f32 DRAM -> staging SBUF, copy into an f32r-rounded tile."""
    sp = stage_pool if stage_pool is not None else pool
    stg = sp.tile(shape, F32, name=f"{name}_s", tag="wstage", bufs=2)
    nc.sync.dma_start(stg[:], src_ap[:])
    dst = pool.tile(shape, F32, name=name, tag=name)
    if eng is nc.scalar:
        nc.scalar.copy(dst.bitcast(F32R), stg[:])
    elif eng is nc.gpsimd:
        nc.gpsimd.tensor_copy(dst.bitcast(F32R), stg[:])
    else:
        nc.vector.tensor_copy(dst.bitcast(F32R), stg[:])
    return dst


def _build_l1(has_gb):
    nc = bacc.Bacc("TRN2", target_bir_lowering=False, debug=False, num_devices=8)
    x_d = nc.dram_tensor("x_sl", [128, NHAL], F32, kind="ExternalInput").ap()
    w1_d = {t: nc.dram_tensor(f"w{t}1T", [128, 128], F32, kind="ExternalInput").ap()
            for t in "qk"}
    w2_d = {t: nc.dram_tensor(f"w{t}2T", [128, 9 * 128], F32, kind="ExternalInput").ap()
            for t in "qk"}
    wv1_d = nc.dram_tensor("wv1T", [128, 128], BF16, kind="ExternalInput").ap()
    wv2_d = nc.dram_tensor("wv2T", [128, 9 * 128], BF16, kind="ExternalInput").ap()
    if has_gb:
        gm_d = nc.dram_tensor("gamma_b", [128, W], F32, kind="ExternalInput").ap()
        bt_d = nc.dram_tensor("beta_b", [128, W], F32, kind="ExternalInput").ap()
    v2_d = nc.dram_tensor("v2o", [128, NPIX], BF16, kind="ExternalOutput").ap()
    q2_d = nc.dram_tensor("q2o", [128, NPIX], F32, kind="ExternalOutput").ap()
    k2_d = nc.dram_tensor("k2o", [128, NPIX], F32, kind="ExternalOutput").ap()

    with tile.TileContext(nc) as tc, ExitStack() as ctx:
        consts = ctx.enter_context(tc.tile_pool(name="consts", bufs=1))
        big = ctx.enter_context(tc.tile_pool(name="big", bufs=1))
        sbw = ctx.enter_context(tc.tile_pool(name="sbw", bufs=3))
        ps_cv = ctx.enter_context(tc.tile_pool(name="ps_cv", bufs=6, space="PSUM"))

        alt = _Alt(nc)

        # x first (one DMA per LN chunk; first chunk is a single group so the
        # conv pipeline head starts early).
        groups = [(g * 4, min(4, RH - g * 4)) for g in range((RH + 3) // 4)]
        NG = len(groups)
        CHUNKS = [[0]] + [[g, g + 1] for g in range(1, NG - 1, 2)]
        g2c = {g: ci for ci, gl in enumerate(CHUNKS) for g in gl}
        x_c, xn_g, xb_g = [], [], []
        w1, w2 = {}, {}
        for ci, gl in enumerate(CHUNKS):
            r0 = groups[gl[0]][0]
            nr = sum(groups[g][1] for g in gl)
            xc = big.tile([128, nr * W], F32, name=f"x{ci}", tag=f"x{ci}")
            nc.sync.dma_start(xc[:], x_d[:, r0 * W:(r0 + nr) * W])
            x_c.append(xc)
            if ci == 0:
                # w1q rides the DMA FIFO right behind x0: conv1 g0 needs it
                w1["q"] = _round_f32r(nc, consts, "w1q", w1_d["q"], [128, 128])
        w1["k"] = _round_f32r(nc, consts, "w1k", w1_d["k"], [128, 128])
        wv1 = consts.tile([128, 128], BF16, name="wv1", tag="wv1")
        nc.sync.dma_start(wv1[:], wv1_d[:])
        for t in "qk":
            w2[t] = _round_f32r(nc, consts, f"w2{t}", w2_d[t], [128, 9 * 128],
                                eng=nc.gpsimd)
        wv2 = consts.tile([128, 9 * 128], BF16, name="wv2", tag="wv2")
        nc.sync.dma_start(wv2[:], wv2_d[:])
        for g, (r0, rows) in enumerate(groups):
            xn_g.append(big.tile([128, rows * W], F32, name=f"xn{g}", tag=f"xn{g}"))
            xb_g.append(big.tile([128, rows * W], BF16, name=f"xb{g}", tag=f"xb{g}"))

        def x_ap(g):
            """AP of group g's rows inside its chunk tile."""
            r0, rows = groups[g]
            ci = g2c[g]
            base = groups[CHUNKS[ci][0]][0]
            off = (r0 - base) * W
            return x_c[ci][:, off:off + rows * W]

        if has_gb:
            gm = consts.tile([128, W], F32, name="gm", tag="gm")
            nc.sync.dma_start(gm[:], gm_d[:])
            bt = consts.tile([128, W], F32, name="bt", tag="bt")
            nc.sync.dma_start(bt[:], bt_d[:])

        epst = sbw.tile([128, 1], F32, name="epst", tag="epst", bufs=1)
        nc.vector.memset(epst[:], EPS)

        # LayerNorm stats via bn_stats in two chunks of groups. The even/odd
        # local-row stride-2 APs can't be dim-merged by lowering, keeping the
        # 128-wide window intact.
        def ln_chunk(ci, glist):
            nrows = sum(groups[g][1] for g in glist)
            bns = sbw.tile([128, nrows, 6], F32, name=f"bns{ci}", tag=f"bns{ci}",
                           bufs=1)
            st = sbw.tile([128, nrows, 8], F32, name=f"st{ci}", tag=f"st{ci}",
                          bufs=1)
            base = groups[glist[0]][0]
            for g in glist:
                r0, rows = groups[g]
                xgv = x_ap(g).rearrange("p (r w) -> p r w", w=W)
                bl = r0 - base
                for lr in range(rows):
                    nc.vector.bn_stats(bns[:, bl + lr, :], xgv[:, lr, :])
            me, mo = bns[:, :, 1:2], bns[:, :, 4:5]
            cve, cvo = bns[:, :, 2:3], bns[:, :, 5:6]
            # shallow-depth combine: ssum=me+mo, t=me^2+mo^2, cv=cve+cvo
            # E[x^2]=cv/W+t/2, var=E[x^2]-ssum^2/4, nmr=-(ssum/2)*rstd
            ssum, t2, cv, var, rstd, nmr = (st[:, :, i:i + 1] for i in range(6))
            tmp = st[:, :, 6:7]
            nc.vector.tensor_tensor(ssum, me, mo, op=ALU.add)
            nc.vector.tensor_tensor(tmp, me, me, op=ALU.mult)
            nc.vector.tensor_tensor(t2, mo, mo, op=ALU.mult)
            nc.vector.tensor_tensor(t2, t2, tmp, op=ALU.add)
            nc.vector.tensor_tensor(cv, cve, cvo, op=ALU.add)
            nc.vector.tensor_scalar_mul(cv, cv, 1.0 / W)
            nc.vector.scalar_tensor_tensor(var, t2, 0.5, cv, op0=ALU.mult,
                                           op1=ALU.add)
            nc.vector.tensor_tensor(tmp, ssum, ssum, op=ALU.mult)
            nc.vector.scalar_tensor_tensor(var, tmp, -0.25, var, op0=ALU.mult,
                                           op1=ALU.add)
            nc.scalar.activation(tmp, var, ACT.Sqrt, bias=epst[:, 0:1])
            nc.vector.reciprocal(rstd, tmp)
            nc.vector.scalar_tensor_tensor(nmr, ssum, -0.5, rstd, op0=ALU.mult,
                                           op1=ALU.mult)
            stf = st.rearrange("p r c -> p (r c)")
            for g in glist:
                r0, rows = groups[g]
                for lr in range(rows):
                    seg = slice(lr * W, (lr + 1) * W)
                    k8 = 8 * (r0 + lr - base)
                    rs = stf[:, k8 + 4:k8 + 5]
                    nm = stf[:, k8 + 5:k8 + 6]
                    xsrc = x_ap(g)[:, seg]
                    if (r0 + lr) % 2 == 0:
                        nc.scalar.activation(xn_g[g][:, seg].bitcast(F32R),
                                             xsrc, ACT.Identity,
                                             bias=nm, scale=rs)
                    else:
                        nc.gpsimd.tensor_scalar(xn_g[g][:, seg].bitcast(F32R),
                                                xsrc, rs, nm,
                                                op0=ALU.mult, op1=ALU.add)
                if has_gb:
                    xnv = xn_g[g].rearrange("p (r w) -> p r w", w=W)
                    gmb = gm.rearrange("p (o w) -> p o w", o=1) \
                        .broadcast_to([128, rows, W])
                    btb = bt.rearrange("p (o w) -> p o w", o=1) \
                        .broadcast_to([128, rows, W])
                    nc.vector.tensor_tensor(xnv.bitcast(F32R), xnv, gmb,
                                            op=ALU.mult)
                    nc.vector.tensor_tensor(xnv.bitcast(F32R), xnv, btb,
                                            op=ALU.add)
                nc.gpsimd.tensor_copy(xb_g[g][:], xn_g[g][:])


        zrow = consts.tile([128, RH], F32, name="zrow", tag="zrow")
        nc.vector.memset(zrow[:], 0.0)
        zr = zrow.rearrange("p (r o) -> p r o", o=1)

        # conv1x1 for all three tensors, group-interleaved. p1 is split into
        # three overlapping row-band tiles per tensor so conv3x3 on the first
        # band can start before conv1x1 finishes (whole-tile dep tracking).
        BANDS = [(0, 13), (12, 25), (24, 33)]   # inclusive row ranges

        def band_of(g):
            return 0 if g <= 2 else (1 if g <= 5 else 2)

        p1 = {}
        for t in "qkv":
            isv = t == "v"
            tiles = []
            for bi, (lo, hi) in enumerate(BANDS):
                nr = hi - lo + 1
                tl = big.tile([128, nr * WP], BF16 if isv else F32,
                              name=f"p1{t}{bi}", tag=f"p1{t}{bi}")
                tv = tl.rearrange("p (r w) -> p r w", w=WP)
                e0 = tv[:, :, 0:1] if isv else tv[:, :, 0:1].bitcast(F32R)
                e1 = (tv[:, :, WP - 1:WP] if isv
                      else tv[:, :, WP - 1:WP].bitcast(F32R))
                nc.vector.tensor_copy(e0, zr[:, :nr, :])
                nc.vector.tensor_copy(e1, zr[:, :nr, :])
                tiles.append(tv)
            p1[t] = tiles
        def conv1_group(g):
            r0, rows = groups[g]
            n = rows * W
            for t in "qkv":
                isv = t == "v"
                ps = ps_cv.tile([128, 512], F32, name=f"cv1{t}{g}", tag="cv")
                if isv:
                    nc.tensor.matmul(ps[:, :n], wv1[:], xb_g[g][:],
                                     start=True, stop=True)
                else:
                    nc.tensor.matmul(ps[:, :n], w1[t].bitcast(F32R),
                                     xn_g[g][:].bitcast(F32R),
                                     start=True, stop=True)
                psv = ps[:, :n].rearrange("p (r w) -> p r w", w=W)

                def _p1dst(bi, a, b, lo=0):
                    d = p1[t][bi][:, a - lo:b - lo, 1:1 + W]
                    return d if isv else d.bitcast(F32R)

                for bi, (lo, hi) in enumerate(BANDS):
                    ra = max(r0, lo)
                    rb = min(r0 + rows, hi + 1)
                    if rb - ra >= 2:
                        rm = (ra + rb) // 2
                        nc.vector.tensor_copy(_p1dst(bi, ra, rm, lo),
                                              psv[:, ra - r0:rm - r0, :])
                        nc.scalar.copy(_p1dst(bi, rm, rb, lo),
                                       psv[:, rm - r0:rb - r0, :])
                    elif ra < rb:
                        alt.copy(_p1dst(bi, ra, rb, lo),
                                 psv[:, ra - r0:rb - r0, :])

        # Emission order == per-engine execution order (the scheduler fixes a
        # static order from priorities), so emit a wavefront that matches the
        # dataflow: LN chunk -> conv1 groups -> conv3 groups as their p1 band
        # completes -> T1 transposes as each conv3 group's c2 tile lands.
        qk_d = {"q": q2_d, "k": k2_d}
        ocur = {}

        def conv3_group(t, g):
            isv = t == "v"
            w2t = wv2 if isv else w2[t]
            bi = band_of(g)
            lo = BANDS[bi][0]
            p1v = p1[t][bi]
            ps2 = ps_cv.tile([128, 512], F32, name=f"cv3{t}{g}", tag="cv")
            for idx in range(9):
                dy, dx = idx // 3, idx % 3
                r = 4 * g + dy - lo
                rhs = p1v[:, r:r + 4, dx:dx + W]
                lhs = w2t[:, idx * 128:(idx + 1) * 128]
                if isv:
                    nc.tensor.matmul(ps2[:], lhs, rhs,
                                     start=(idx == 0), stop=(idx == 8))
                else:
                    nc.tensor.matmul(ps2[:], lhs.bitcast(F32R),
                                     rhs.bitcast(F32R),
                                     start=(idx == 0), stop=(idx == 8))
            half = slice(512 * (g % 2), 512 * (g % 2) + 512)
            if isv:
                if g % 2 == 0:
                    ocur["v"] = sbw.tile([128, 1024], BF16, name=f"vch{g}",
                                         tag="vch", bufs=2)
                alt.copy(ocur["v"][:, half], ps2[:])
                if g % 2 == 1:
                    nc.sync.dma_start(v2_d[:, (g - 1) * 512:(g + 1) * 512],
                                      ocur["v"][:])
            else:
                if g % 2 == 0:
                    ocur[t] = sbw.tile([128, 1024], F32, name=f"c2{t}{g}",
                                       tag=f"c2{t}", bufs=2)
                alt.copy(ocur[t][:, half], ps2[:])
                if g % 2 == 1:
                    nc.sync.dma_start(qk_d[t][:, (g - 1) * 512:(g + 1) * 512],
                                      ocur[t][:])

        ln_chunk(0, CHUNKS[0])
        conv1_group(0)
        ln_chunk(1, CHUNKS[1])
        conv1_group(1)
        conv1_group(2)
        ln_chunk(2, CHUNKS[2])
        conv1_group(3)
        conv1_group(4)
        ln_chunk(3, CHUNKS[3])
        ln_chunk(4, CHUNKS[4])
        for g in (0, 1, 2):          # band A ready after conv1 g3
            for t in "qkv":
                conv3_group(t, g)
        conv1_group(5)
        conv1_group(6)
        conv1_group(7)
        conv1_group(8)
        for g in (3, 4, 5):          # band B ready after conv1 g6
            for t in "qkv":
                conv3_group(t, g)
        for g in (6, 7):             # band C ready after conv1 g8
            for t in "qkv":
                conv3_group(t, g)

    nc.compile()
    return nc


L2IN = 256 + 2 * NPIX   # pstack | woT | per-chunk (vr | xr) interleave


def _build_l2():
    nc = bacc.Bacc("TRN2", target_bir_lowering=False, debug=False, num_devices=8)
    in_d = nc.dram_tensor("l2in", [128, L2IN], BF16, kind="ExternalInput").ap()
    y_d = nc.dram_tensor("y_sl", [128, NPIX], BF16, kind="ExternalOutput").ap()

    with tile.TileContext(nc) as tc, ExitStack() as ctx:
        consts = ctx.enter_context(tc.tile_pool(name="consts", bufs=1))
        big = ctx.enter_context(tc.tile_pool(name="big", bufs=1))
        sbw = ctx.enter_context(tc.tile_pool(name="sbw", bufs=3))
        ps_a = ctx.enter_context(tc.tile_pool(name="ps_a", bufs=4, space="PSUM"))

        alt = _Alt(nc, act_every=1)

        hw = consts.tile([128, 256], BF16, name="hw0", tag="hw0")
        nc.sync.dma_start(hw[:], in_d[:, 0:256])
        pst = hw[:, 0:128]
        wo = hw[:, 128:256]
        chunks = {}

        def fetch(c):
            if c >= 4 or c in chunks:
                return
            t = big.tile([128, 2048], BF16, name=f"ch{c}", tag=f"ch{c}")
            nc.sync.dma_start(t[:], in_d[:, 256 + 2048 * c:256 + 2048 * (c + 1)])
            chunks[c] = t

        fetch(0)
        fetch(1)
        for c in range(4):
            ch = chunks[c]
            ysb = sbw.tile([128, 1024], BF16, name=f"ysb{c}", tag=f"ysb{c}",
                           bufs=1)
            for h in range(2):
                sl = slice(h * 512, (h + 1) * 512)
                xsl = slice(1024 + h * 512, 1024 + (h + 1) * 512)
                ps = ps_a.tile([128, 512], F32, name=f"ar{c}{h}", tag="ar",
                               bufs=2)
                nc.tensor.matmul(ps[:], pst, ch[:, sl], start=True, stop=True)
                oat = sbw.tile([128, 512], BF16, name=f"oat{c}{h}", tag="oat",
                               bufs=3)
                alt.copy(oat[:], ps[:])
                ps4 = ps_a.tile([128, 512], F32, name=f"fin{c}{h}", tag="fin",
                                bufs=2)
                nc.tensor.matmul(ps4[:], wo, oat[:], start=True, stop=True)
                nc.vector.tensor_tensor(ysb[:, sl], ps4[:], ch[:, xsl],
                                        op=ALU.add)
            nc.sync.dma_start(y_d[:, c * 1024:(c + 1) * 1024], ysb[:])
            fetch(c + 2)

    nc.compile()
    return nc


def _get(name, has_gb=False):
    key = (name, has_gb)
    if key not in _CACHE:
        _CACHE[key] = _build_l1(has_gb) if name == "l1" else _build_l2()
    return _CACHE[key]


def _host_middle(q2_list, k2_list, v2o_list, scale):
    """q2/k2 -> gram -> softmax P + PSTACK; v2o -> per-core VROWS (bf16)."""
    f = np.float32
    G = np.zeros((B, HEADS, D, D), f)
    for c in range(8):
        # [c=(h,ci), (y,xs,d)] -> [h, ci, y, xs, d]
        qv = q2_list[c].reshape(HEADS, 16, RPC, 8, D)
        kv = k2_list[c].reshape(HEADS, 16, RPC, 8, D)
        G[c // 4] += np.einsum("hcyxd,hcyxj->hdj", kv, qv, optimize=True)
    G /= float(np.asarray(scale, f)[0])
    Gm = G - G.max(-1, keepdims=True)
    E = np.exp(Gm)
    P = (E / E.sum(-1, keepdims=True)).astype(f)          # [B, HEADS, 16, 16]

    pstack = np.zeros((B, 128, 128), f)
    for b in range(B):
        for hd in range(HEADS):
            pstack[b, 16 * hd:16 * hd + 16, 16 * hd:16 * hd + 16] = P[b, hd]

    v_conv = np.empty((B, C, H, W), NBF)
    for c in range(8):
        b, r0 = c // 4, 32 * (c % 4)
        v_conv[b, :, r0:r0 + RPC, :] = v2o_list[c].reshape(C, RPC, W)
    # vc[b, hd, ci, y, xs, d]
    vc = v_conv.reshape(B, HEADS, 16, H, 8, 16)
    vrows = []
    for c in range(8):
        b, r0 = c // 4, 32 * (c % 4)
        rows = np.empty((32, 128, 128), NBF)
        for i in range(32):
            yp = r0 + i
            ci, yb = yp // 8, yp % 8
            blk = vc[b, :, ci, 16 * yb:16 * yb + 16, :, :]   # [hd, yy, xs, d]
            rows[i] = blk.transpose(0, 3, 1, 2).reshape(128, 128)
        vrows.append(np.ascontiguousarray(rows.transpose(1, 0, 2))
                     .reshape(128, NPIX))
    return pstack.astype(NBF), vrows


def _maps_l1(x, gamma, beta, wq1, wq2, wk1, wk2, wv1, wv2, has_gb):
    f = np.float32
    xp = np.pad(np.asarray(x, f), ((0, 0), (0, 0), (1, 1), (0, 0)))
    common = {}
    if has_gb:
        common["gamma_b"] = np.broadcast_to(np.asarray(gamma, f), (128, W)).copy()
        common["beta_b"] = np.broadcast_to(np.asarray(beta, f), (128, W)).copy()
    for t, w1_, w2_ in (("q", wq1, wq2), ("k", wk1, wk2)):
        common[f"w{t}1T"] = np.ascontiguousarray(np.asarray(w1_, f)[:, :, 0, 0].T)
        common[f"w{t}2T"] = np.ascontiguousarray(
            np.asarray(w2_, f).transpose(1, 2, 3, 0).reshape(128, 9 * 128))
    common["wv1T"] = np.ascontiguousarray(
        np.asarray(wv1, f)[:, :, 0, 0].T).astype(NBF)
    common["wv2T"] = np.ascontiguousarray(
        np.asarray(wv2, f).transpose(1, 2, 3, 0).reshape(128, 9 * 128)).astype(NBF)
    maps = []
    for c in range(8):
        b, r0 = c // 4, 32 * (c % 4)
        m = dict(common)
        m["x_sl"] = np.ascontiguousarray(xp[b, :, r0:r0 + RH, :].reshape(128, NHAL))
        maps.append(m)
    return maps


def _run(nc, maps, key):
    trace = bool(int(os.environ.get("KERNEL_TRACE", "0")))
    if _CACHE.get("sim"):
        from concourse.bass_interp import MultiCoreSim
        sim = MultiCoreSim(nc, num_cores=8, require_finite=True, require_nnan=True)
        cores = list(sim.cores.values())
        for c, m in enumerate(maps):
            for k, v in m.items():
                cores[c].tensor(k)[:] = v
        sim.simulate(check_with_hw=False)
        return [{k: np.array(cores[c].tensor(k)) for k in key} for c in range(8)]
    res = bass_utils.run_bass_kernel_spmd(nc, maps, core_ids=list(range(8)),
                                          trace=trace)
    _CACHE.setdefault("results", []).append(res)
    return res.results


def kernel(x, gamma, beta, scale, wq1, wq2, wk1, wk2, wv1, wv2, wo):
    f = np.float32
    has_gb = not (np.allclose(np.asarray(gamma, f), 1.0)
                  and np.allclose(np.asarray(beta, f), 0.0))
    r1 = _run(_get("l1", has_gb),
              _maps_l1(x, gamma, beta, wq1, wq2, wk1, wk2, wv1, wv2, has_gb),
              ("v2o", "q2o", "k2o"))
    pstack, vrows = _host_middle([r["q2o"] for r in r1],
                                 [r["k2o"] for r in r1],
                                 [r["v2o"] for r in r1], scale)
    woT = np.ascontiguousarray(np.asarray(wo, f)[:, :, 0, 0].T).astype(NBF)
    xf = np.asarray(x, f)
    maps2 = []
    for c in range(8):
        b, r0 = c // 4, 32 * (c % 4)
        xr = xf[b, :, r0:r0 + RPC, :].reshape(128, NPIX).astype(NBF)
        vrc = vrows[c]
        l2in = np.empty((128, L2IN), NBF)
        l2in[:, 0:128] = pstack[b]
        l2in[:, 128:256] = woT
        for cc in range(4):
            base = 256 + 2048 * cc
            l2in[:, base:base + 1024] = vrc[:, 1024 * cc:1024 * (cc + 1)]
            l2in[:, base + 1024:base + 2048] = xr[:, 1024 * cc:1024 * (cc + 1)]
        maps2.append({"l2in": l2in})
    r2 = _run(_get("l2"), maps2, ("y_sl",))
    y = np.empty((B, C, H, W), f)
    for c in range(8):
        b, r0 = c // 4, 32 * (c % 4)
        y[b, :, r0:r0 + RPC, :] = r2[c]["y_sl"].reshape(C, RPC, W).astype(f)
    return y


def kernel_sim(**inputs):
    _CACHE["sim"] = True
    try:
        return kernel(**inputs)
    finally:
        _CACHE["sim"] = False


# revision 51
# speedup vs baseline: 1.0024x; 1.0024x over previous
"""MDTA Bass kernel for 8 TRN2 NeuronCores, two SPMD launches (optimized).

Reference semantics (row-major reshape!): q.reshape(B,HEADS,HW,D) maps
  tensor[b,hd,s,d] = conv[b, 16*hd+ci, y, 16*xs+d]  with s = ci*1024+y*8+xs
so the attention feature axis d is the LOW 4 BITS OF x, and
  attn[hd,d,j] = sum_{ci,y,xs} k2[16hd+ci,y,16xs+d]*q2[16hd+ci,y,16xs+j]
  out_conv[b,16hd+j, ci*8+y//16, (y%16)*8+xs] = sum_d v2[16hd+ci,y,16xs+d]*P[d,j]

Launch 1 (spatial shards: b x quarter-of-H, 1-row halo): LayerNorm via
bn_stats, the three conv1x1+conv3x3 chains (q,k in fp32r; v fully bf16),
and per-head 128x128 "pair" matrices via two levels of PE transposes.
Host: extracts strip-diagonals -> gram, softmax -> P, builds block-diag
PSTACK and per-output-row VROW (bf16) rearrangements of v. Launch 2:
P-apply + wo-conv1x1 (bf16 matmuls) + fp32 residual.

PSUM->SBUF copies alternate DVE/ACT; the bf16 cast of xn runs on gpsimd.
"""

import os
from contextlib import ExitStack

import numpy as np
import ml_dtypes

import concourse.bacc as bacc
import concourse.bass as bass
import concourse.mybir as mybir
import concourse.tile as tile
from concourse import bass_utils

F32 = mybir.dt.float32
WARM = int(os.environ.get('KWARM', '0'))
F32R = mybir.dt.float32r
BF16 = mybir.dt.bfloat16
AX = mybir.AxisListType
ALU = mybir.AluOpType
ACT = mybir.ActivationFunctionType

B, C, H, W = 2, 128, 128, 128
HEADS, D = 8, 16
EPS = 1e-5
RPC = H // 4
RH = RPC + 2
NPIX = RPC * W          # 4096
NHAL = RH * W           # 4352
WP = W + 2

NBF = ml_dtypes.bfloat16

_CACHE = {}


class _Alt:
    """Alternate copy engines: mostly DVE, every `act_every`-th op on ACT."""

    def __init__(self, nc, act_every=None):
        self.nc = nc
        self.i = 0
        if act_every is None:
            act_every = int(os.environ.get("KALT", "3"))
        self.act_every = act_every

    def copy(self, dst, src):
        self.i += 1
        if self.i % self.act_every == 0:
            self.nc.scalar.copy(dst, src)
        else:
            self.nc.vector.tensor_copy(dst, src)


def _round_f32r(nc, pool, name, src_ap, shape, stage_pool=None, eng=None):
    """DMA f32 DRAM -> staging SBUF, copy into an f32r-rounded tile."""
    sp = stage_pool if stage_pool is not None else pool
    stg = sp.tile(shape, F32, name=f"{name}_s", tag="wstage", bufs=2)
    nc.sync.dma_start(stg[:], src_ap[:])
    dst = pool.tile(shape, F32, name=name, tag=name)
    if eng is nc.scalar:
        nc.scalar.copy(dst.bitcast(F32R), stg[:])
    elif eng is nc.gpsimd:
        nc.gpsimd.tensor_copy(dst.bitcast(F32R), stg[:])
    else:
        nc.vector.tensor_copy(dst.bitcast(F32R), stg[:])
    return dst


def _build_l1(has_gb):
    nc = bacc.Bacc("TRN2", target_bir_lowering=False, debug=False, num_devices=8)
    x_d = nc.dram_tensor("x_sl", [128, NHAL], F32, kind="ExternalInput").ap()
    w1_d = {t: nc.dram_tensor(f"w{t}1T", [128, 128], F32, kind="ExternalInput").ap()
            for t in "qk"}
    w2_d = {t: nc.dram_tensor(f"w{t}2T", [128, 9 * 128], F32, kind="ExternalInput").ap()
            for t in "qk"}
    wv1_d = nc.dram_tensor("wv1T", [128, 128], BF16, kind="ExternalInput").ap()
    wv2_d = nc.dram_tensor("wv2T", [128, 9 * 128], BF16, kind="ExternalInput").ap()
    if has_gb:
        gm_d = nc.dram_tensor("gamma_b", [128, W], F32, kind="ExternalInput").ap()
        bt_d = nc.dram_tensor("beta_b", [128, W], F32, kind="ExternalInput").ap()
    v2_d = nc.dram_tensor("v2o", [128, NPIX], BF16, kind="ExternalOutput").ap()
    q2_d = nc.dram_tensor("q2o", [128, NPIX], F32, kind="ExternalOutput").ap()
    k2_d = nc.dram_tensor("k2o", [128, NPIX], F32, kind="ExternalOutput").ap()

    with tile.TileContext(nc) as tc, ExitStack() as ctx:
        consts = ctx.enter_context(tc.tile_pool(name="consts", bufs=1))
        big = ctx.enter_context(tc.tile_pool(name="big", bufs=1))
        sbw = ctx.enter_context(tc.tile_pool(name="sbw", bufs=3))
        ps_cv = ctx.enter_context(tc.tile_pool(name="ps_cv", bufs=6, space="PSUM"))

        alt = _Alt(nc)

        # x first (one DMA per LN chunk; first chunk is a single group so the
        # conv pipeline head starts early).
        groups = [(g * 4, min(4, RH - g * 4)) for g in range((RH + 3) // 4)]
        NG = len(groups)
        CHUNKS = [[0]] + [[g, g + 1] for g in range(1, NG - 1, 2)]
        g2c = {g: ci for ci, gl in enumerate(CHUNKS) for g in gl}
        x_c, xn_g, xb_g = [], [], []
        w1, w2 = {}, {}
        for ci, gl in enumerate(CHUNKS):
            r0 = groups[gl[0]][0]
            nr = sum(groups[g][1] for g in gl)
            xc = big.tile([128, nr * W], F32, name=f"x{ci}", tag=f"x{ci}")
            nc.sync.dma_start(xc[:], x_d[:, r0 * W:(r0 + nr) * W])
            x_c.append(xc)
            if ci == 0:
                # w1q rides the DMA FIFO right behind x0: conv1 g0 needs it
                w1["q"] = _round_f32r(nc, consts, "w1q", w1_d["q"], [128, 128])
        w1["k"] = _round_f32r(nc, consts, "w1k", w1_d["k"], [128, 128])
        wv1 = consts.tile([128, 128], BF16, name="wv1", tag="wv1")
        nc.sync.dma_start(wv1[:], wv1_d[:])
        for t in "qk":
            w2[t] = _round_f32r(nc, consts, f"w2{t}", w2_d[t], [128, 9 * 128],
                                eng=nc.gpsimd)
        wv2 = consts.tile([128, 9 * 128], BF16, name="wv2", tag="wv2")
        nc.sync.dma_start(wv2[:], wv2_d[:])
        for g, (r0, rows) in enumerate(groups):
            xn_g.append(big.tile([128, rows * W], F32, name=f"xn{g}", tag=f"xn{g}"))
            xb_g.append(big.tile([128, rows * W], BF16, name=f"xb{g}", tag=f"xb{g}"))

        def x_ap(g):
            """AP of group g's rows inside its chunk tile."""
            r0, rows = groups[g]
            ci = g2c[g]
            base = groups[CHUNKS[ci][0]][0]
            off = (r0 - base) * W
            return x_c[ci][:, off:off + rows * W]

        if has_gb:
            gm = consts.tile([128, W], F32, name="gm", tag="gm")
            nc.sync.dma_start(gm[:], gm_d[:])
            bt = consts.tile([128, W], F32, name="bt", tag="bt")
            nc.sync.dma_start(bt[:], bt_d[:])

        epst = sbw.tile([128, 1], F32, name="epst", tag="epst", bufs=1)
        nc.vector.memset(epst[:], EPS)

        # PE warm-up: ~3us of throwaway matmuls while LN runs, so the HAM
        # clock is at full rate when the real convs arrive.
        ps_w = ctx.enter_context(tc.tile_pool(name="ps_w", bufs=1, space="PSUM"))
        wsrc = w1["q"]
        for i in range(WARM):
            psw = ps_w.tile([128, 128], F32, name=f"warm{i}", tag="warm")
            nc.tensor.matmul(psw[:], wsrc.bitcast(F32R), wsrc.bitcast(F32R),
                             start=True, stop=True)

        # LayerNorm stats via bn_stats in two chunks of groups. The even/odd
        # local-row stride-2 APs can't be dim-merged by lowering, keeping the
        # 128-wide window intact.
        def ln_chunk(ci, glist):
            nrows = sum(groups[g][1] for g in glist)
            bns = sbw.tile([128, nrows, 6], F32, name=f"bns{ci}", tag=f"bns{ci}",
                           bufs=1)
            st = sbw.tile([128, nrows, 8], F32, name=f"st{ci}", tag=f"st{ci}",
                          bufs=1)
            base = groups[glist[0]][0]
            for g in glist:
                r0, rows = groups[g]
                xgv = x_ap(g).rearrange("p (r w) -> p r w", w=W)
                bl = r0 - base
                for lr in range(rows):
                    nc.vector.bn_stats(bns[:, bl + lr, :], xgv[:, lr, :])
            me, mo = bns[:, :, 1:2], bns[:, :, 4:5]
            cve, cvo = bns[:, :, 2:3], bns[:, :, 5:6]
            # shallow-depth combine: ssum=me+mo, t=me^2+mo^2, cv=cve+cvo
            # E[x^2]=cv/W+t/2, var=E[x^2]-ssum^2/4, nmr=-(ssum/2)*rstd
            ssum, t2, cv, var, rstd, nmr = (st[:, :, i:i + 1] for i in range(6))
            tmp = st[:, :, 6:7]
            nc.vector.tensor_tensor(ssum, me, mo, op=ALU.add)
            nc.vector.tensor_tensor(tmp, me, me, op=ALU.mult)
            nc.vector.tensor_tensor(t2, mo, mo, op=ALU.mult)
            nc.vector.tensor_tensor(t2, t2, tmp, op=ALU.add)
            nc.vector.tensor_tensor(cv, cve, cvo, op=ALU.add)
            nc.vector.tensor_scalar_mul(cv, cv, 1.0 / W)
            nc.vector.scalar_tensor_tensor(var, t2, 0.5, cv, op0=ALU.mult,
                                           op1=ALU.add)
            nc.vector.tensor_tensor(tmp, ssum, ssum, op=ALU.mult)
            nc.vector.scalar_tensor_tensor(var, tmp, -0.25, var, op0=ALU.mult,
                                           op1=ALU.add)
            nc.scalar.activation(tmp, var, ACT.Sqrt, bias=epst[:, 0:1])
            nc.vector.reciprocal(rstd, tmp)
            nc.vector.scalar_tensor_tensor(nmr, ssum, -0.5, rstd, op0=ALU.mult,
                                           op1=ALU.mult)
            stf = st.rearrange("p r c -> p (r c)")
            for g in glist:
                r0, rows = groups[g]
                for lr in range(rows):
                    seg = slice(lr * W, (lr + 1) * W)
                    k8 = 8 * (r0 + lr - base)
                    rs = stf[:, k8 + 4:k8 + 5]
                    nm = stf[:, k8 + 5:k8 + 6]
                    xsrc = x_ap(g)[:, seg]
                    if (r0 + lr) % 2 == 0:
                        nc.scalar.activation(xn_g[g][:, seg].bitcast(F32R),
                                             xsrc, ACT.Identity,
                                             bias=nm, scale=rs)
                    else:
                        nc.gpsimd.tensor_scalar(xn_g[g][:, seg].bitcast(F32R),
                                                xsrc, rs, nm,
                                                op0=ALU.mult, op1=ALU.add)
                if has_gb:
                    xnv = xn_g[g].rearrange("p (r w) -> p r w", w=W)
                    gmb = gm.rearrange("p (o w) -> p o w", o=1) \
                        .broadcast_to([128, rows, W])
                    btb = bt.rearrange("p (o w) -> p o w", o=1) \
                        .broadcast_to([128, rows, W])
                    nc.vector.tensor_tensor(xnv.bitcast(F32R), xnv, gmb,
                                            op=ALU.mult)
                    nc.vector.tensor_tensor(xnv.bitcast(F32R), xnv, btb,
                                            op=ALU.add)
                nc.gpsimd.tensor_copy(xb_g[g][:], xn_g[g][:])


        zrow = consts.tile([128, RH], F32, name="zrow", tag="zrow")
        nc.vector.memset(zrow[:], 0.0)
        zr = zrow.rearrange("p (r o) -> p r o", o=1)

        # conv1x1 for all three tensors, group-interleaved. p1 is split into
        # three overlapping row-band tiles per tensor so conv3x3 on the first
        # band can start before conv1x1 finishes (whole-tile dep tracking).
        BANDS = [(0, 13), (12, 25), (24, 33)]   # inclusive row ranges

        def band_of(g):
            return 0 if g <= 2 else (1 if g <= 5 else 2)

        p1 = {}
        for t in "qkv":
            isv = t == "v"
            tiles = []
            for bi, (lo, hi) in enumerate(BANDS):
                nr = hi - lo + 1
                tl = big.tile([128, nr * WP], BF16 if isv else F32,
                              name=f"p1{t}{bi}", tag=f"p1{t}{bi}")
                tv = tl.rearrange("p (r w) -> p r w", w=WP)
                e0 = tv[:, :, 0:1] if isv else tv[:, :, 0:1].bitcast(F32R)
                e1 = (tv[:, :, WP - 1:WP] if isv
                      else tv[:, :, WP - 1:WP].bitcast(F32R))
                nc.vector.tensor_copy(e0, zr[:, :nr, :])
                nc.vector.tensor_copy(e1, zr[:, :nr, :])
                tiles.append(tv)
            p1[t] = tiles
        def conv1_group(g):
            r0, rows = groups[g]
            n = rows * W
            for t in "qkv":
                isv = t == "v"
                ps = ps_cv.tile([128, 512], F32, name=f"cv1{t}{g}", tag="cv")
                if isv:
                    nc.tensor.matmul(ps[:, :n], wv1[:], xb_g[g][:],
                                     start=True, stop=True)
                else:
                    nc.tensor.matmul(ps[:, :n], w1[t].bitcast(F32R),
                                     xn_g[g][:].bitcast(F32R),
                                     start=True, stop=True)
                psv = ps[:, :n].rearrange("p (r w) -> p r w", w=W)

                def _p1dst(bi, a, b, lo=0):
                    d = p1[t][bi][:, a - lo:b - lo, 1:1 + W]
                    return d if isv else d.bitcast(F32R)

                for bi, (lo, hi) in enumerate(BANDS):
                    ra = max(r0, lo)
                    rb = min(r0 + rows, hi + 1)
                    if rb - ra >= 2:
                        rm = (ra + rb) // 2
                        nc.vector.tensor_copy(_p1dst(bi, ra, rm, lo),
                                              psv[:, ra - r0:rm - r0, :])
                        nc.scalar.copy(_p1dst(bi, rm, rb, lo),
                                       psv[:, rm - r0:rb - r0, :])
                    elif ra < rb:
                        alt.copy(_p1dst(bi, ra, rb, lo),
                                 psv[:, ra - r0:rb - r0, :])

        # Emission order == per-engine execution order (the scheduler fixes a
        # static order from priorities), so emit a wavefront that matches the
        # dataflow: LN chunk -> conv1 groups -> conv3 groups as their p1 band
        # completes -> T1 transposes as each conv3 group's c2 tile lands.
        qk_d = {"q": q2_d, "k": k2_d}
        ocur = {}

        def conv3_group(t, g):
            isv = t == "v"
            w2t = wv2 if isv else w2[t]
            bi = band_of(g)
            lo = BANDS[bi][0]
            p1v = p1[t][bi]
            ps2 = ps_cv.tile([128, 512], F32, name=f"cv3{t}{g}", tag="cv")
            for idx in range(9):
                dy, dx = idx // 3, idx % 3
                r = 4 * g + dy - lo
                rhs = p1v[:, r:r + 4, dx:dx + W]
                lhs = w2t[:, idx * 128:(idx + 1) * 128]
                if isv:
                    nc.tensor.matmul(ps2[:], lhs, rhs,
                                     start=(idx == 0), stop=(idx == 8))
                else:
                    nc.tensor.matmul(ps2[:], lhs.bitcast(F32R),
                                     rhs.bitcast(F32R),
                                     start=(idx == 0), stop=(idx == 8))
            out_d = v2_d if isv else qk_d[t]
            key = "v" if isv else t
            if g >= 6:
                # tail groups: single-512 DMAs so the last transfer starts asap
                dt_o = BF16 if isv else F32
                oc = sbw.tile([128, 512], dt_o, name=f"o{key}{g}",
                              tag=f"ot{key}", bufs=2)
                alt.copy(oc[:], ps2[:])
                nc.sync.dma_start(out_d[:, g * 512:(g + 1) * 512], oc[:])
            else:
                half = slice(512 * (g % 2), 512 * (g % 2) + 512)
                if g % 2 == 0:
                    dt_o = BF16 if isv else F32
                    ocur[key] = sbw.tile([128, 1024], dt_o, name=f"c2{key}{g}",
                                         tag=f"c2{key}", bufs=2)
                alt.copy(ocur[key][:, half], ps2[:])
                if g % 2 == 1:
                    nc.sync.dma_start(out_d[:, (g - 1) * 512:(g + 1) * 512],
                                      ocur[key][:])

        ln_chunk(0, CHUNKS[0])
        conv1_group(0)
        ln_chunk(1, CHUNKS[1])
        conv1_group(1)
        conv1_group(2)
        ln_chunk(2, CHUNKS[2])
        conv1_group(3)
        conv1_group(4)
        ln_chunk(3, CHUNKS[3])
        ln_chunk(4, CHUNKS[4])
        for g in (0, 1, 2):          # band A ready after conv1 g3
            for t in "qkv":
                conv3_group(t, g)
        conv1_group(5)
        conv1_group(6)
        conv1_group(7)
        conv1_group(8)
        for g in (3, 4, 5):          # band B ready after conv1 g6
            for t in "qkv":
                conv3_group(t, g)
        for g in (6, 7):             # band C ready after conv1 g8
            for t in "qkv":
                conv3_group(t, g)

    nc.compile()
    return nc


L2IN = 256 + 2 * NPIX   # pstack | woT | per-chunk (vr | xr) interleave


def _build_l2():
    nc = bacc.Bacc("TRN2", target_bir_lowering=False, debug=False, num_devices=8)
    in_d = nc.dram_tensor("l2in", [128, L2IN], BF16, kind="ExternalInput").ap()
    y_d = nc.dram_tensor("y_sl", [128, NPIX], BF16, kind="ExternalOutput").ap()

    with tile.TileContext(nc) as tc, ExitStack() as ctx:
        consts = ctx.enter_context(tc.tile_pool(name="consts", bufs=1))
        big = ctx.enter_context(tc.tile_pool(name="big", bufs=1))
        sbw = ctx.enter_context(tc.tile_pool(name="sbw", bufs=3))
        ps_a = ctx.enter_context(tc.tile_pool(name="ps_a", bufs=4, space="PSUM"))

        alt = _Alt(nc, act_every=1)

        hw = consts.tile([128, 256], BF16, name="hw0", tag="hw0")
        nc.sync.dma_start(hw[:], in_d[:, 0:256])
        pst = hw[:, 0:128]
        wo = hw[:, 128:256]
        chunks = {}

        def fetch(c):
            if c >= 4 or c in chunks:
                return
            t = big.tile([128, 2048], BF16, name=f"ch{c}", tag=f"ch{c}")
            nc.sync.dma_start(t[:], in_d[:, 256 + 2048 * c:256 + 2048 * (c + 1)])
            chunks[c] = t

        fetch(0)
        fetch(1)
        for c in range(4):
            ch = chunks[c]
            ysb = sbw.tile([128, 1024], BF16, name=f"ysb{c}", tag=f"ysb{c}",
                           bufs=1)
            for h in range(2):
                sl = slice(h * 512, (h + 1) * 512)
                xsl = slice(1024 + h * 512, 1024 + (h + 1) * 512)
                ps = ps_a.tile([128, 512], F32, name=f"ar{c}{h}", tag="ar",
                               bufs=2)
                nc.tensor.matmul(ps[:], pst, ch[:, sl], start=True, stop=True)
                oat = sbw.tile([128, 512], BF16, name=f"oat{c}{h}", tag="oat",
                               bufs=3)
                alt.copy(oat[:], ps[:])
                ps4 = ps_a.tile([128, 512], F32, name=f"fin{c}{h}", tag="fin",
                                bufs=2)
                nc.tensor.matmul(ps4[:], wo, oat[:], start=True, stop=True)
                nc.vector.tensor_tensor(ysb[:, sl], ps4[:], ch[:, xsl],
                                        op=ALU.add)
            nc.sync.dma_start(y_d[:, c * 1024:(c + 1) * 1024], ysb[:])
            fetch(c + 2)

    nc.compile()
    return nc


def _get(name, has_gb=False):
    key = (name, has_gb)
    if key not in _CACHE:
        _CACHE[key] = _build_l1(has_gb) if name == "l1" else _build_l2()
    return _CACHE[key]


def _host_middle(q2_list, k2_list, v2o_list, scale):
    """q2/k2 -> gram -> softmax P + PSTACK; v2o -> per-core VROWS (bf16)."""
    f = np.float32
    G = np.zeros((B, HEADS, D, D), f)
    for c in range(8):
        # [c=(h,ci), (y,xs,d)] -> [h, ci, y, xs, d]
        qv = q2_list[c].reshape(HEADS, 16, RPC, 8, D)
        kv = k2_list[c].reshape(HEADS, 16, RPC, 8, D)
        G[c // 4] += np.einsum("hcyxd,hcyxj->hdj", kv, qv, optimize=True)
    G /= float(np.asarray(scale, f)[0])
    Gm = G - G.max(-1, keepdims=True)
    E = np.exp(Gm)
    P = (E / E.sum(-1, keepdims=True)).astype(f)          # [B, HEADS, 16, 16]

    pstack = np.zeros((B, 128, 128), f)
    for b in range(B):
        for hd in range(HEADS):
            pstack[b, 16 * hd:16 * hd + 16, 16 * hd:16 * hd + 16] = P[b, hd]

    v_conv = np.empty((B, C, H, W), NBF)
    for c in range(8):
        b, r0 = c // 4, 32 * (c % 4)
        v_conv[b, :, r0:r0 + RPC, :] = v2o_list[c].reshape(C, RPC, W)
    # vc[b, hd, ci, y, xs, d]
    vc = v_conv.reshape(B, HEADS, 16, H, 8, 16)
    vrows = []
    for c in range(8):
        b, r0 = c // 4, 32 * (c % 4)
        rows = np.empty((32, 128, 128), NBF)
        for i in range(32):
            yp = r0 + i
            ci, yb = yp // 8, yp % 8
            blk = vc[b, :, ci, 16 * yb:16 * yb + 16, :, :]   # [hd, yy, xs, d]
            rows[i] = blk.transpose(0, 3, 1, 2).reshape(128, 128)
        vrows.append(np.ascontiguousarray(rows.transpose(1, 0, 2))
                     .reshape(128, NPIX))
    return pstack.astype(NBF), vrows


def _maps_l1(x, gamma, beta, wq1, wq2, wk1, wk2, wv1, wv2, has_gb):
    f = np.float32
    xp = np.pad(np.asarray(x, f), ((0, 0), (0, 0), (1, 1), (0, 0)))
    common = {}
    if has_gb:
        common["gamma_b"] = np.broadcast_to(np.asarray(gamma, f), (128, W)).copy()
        common["beta_b"] = np.broadcast_to(np.asarray(beta, f), (128, W)).copy()
    for t, w1_, w2_ in (("q", wq1, wq2), ("k", wk1, wk2)):
        common[f"w{t}1T"] = np.ascontiguousarray(np.asarray(w1_, f)[:, :, 0, 0].T)
        common[f"w{t}2T"] = np.ascontiguousarray(
            np.asarray(w2_, f).transpose(1, 2, 3, 0).reshape(128, 9 * 128))
    common["wv1T"] = np.ascontiguousarray(
        np.asarray(wv1, f)[:, :, 0, 0].T).astype(NBF)
    common["wv2T"] = np.ascontiguousarray(
        np.asarray(wv2, f).transpose(1, 2, 3, 0).reshape(128, 9 * 128)).astype(NBF)
    maps = []
    for c in range(8):
        b, r0 = c // 4, 32 * (c % 4)
        m = dict(common)
        m["x_sl"] = np.ascontiguousarray(xp[b, :, r0:r0 + RH, :].reshape(128, NHAL))
        maps.append(m)
    return maps


def _run(nc, maps, key):
    trace = bool(int(os.environ.get("KERNEL_TRACE", "0")))
    if _CACHE.get("sim"):
        from concourse.bass_interp import MultiCoreSim
        sim = MultiCoreSim(nc, num_cores=8, require_finite=True, require_nnan=True)
        cores = list(sim.cores.values())
        for c, m in enumerate(maps):
            for k, v in m.items():
                cores[c].tensor(k)[:] = v
        sim.simulate(check_with_hw=False)
        return [{k: np.array(cores[c].tensor(k)) for k in key} for c in range(8)]
    res = bass_utils.run_bass_kernel_spmd(nc, maps, core_ids=list(range(8)),
                                          trace=trace)
    _CACHE.setdefault("results", []).append(res)
    return res.results


def kernel(x, gamma, beta, scale, wq1, wq2, wk1, wk2, wv1, wv2, wo):
    f = np.float32
    has_gb = not (np.allclose(np.asarray(gamma, f), 1.0)
                  and np.allclose(np.asarray(beta, f), 0.0))
    r1 = _run(_get("l1", has_gb),
              _maps_l1(x, gamma, beta, wq1, wq2, wk1, wk2, wv1, wv2, has_gb),
              ("v2o", "q2o", "k2o"))
    pstack, vrows = _host_middle([r["q2o"] for r in r1],
                                 [r["k2o"] for r in r1],
                                 [r["v2o"] for r in r1], scale)
    woT = np.ascontiguousarray(np.asarray(wo, f)[:, :, 0, 0].T).astype(NBF)
    xf = np.asarray(x, f)
    maps2 = []
    for c in range(8):
        b, r0 = c // 4, 32 * (c % 4)
        xr = xf[b, :, r0:r0 + RPC, :].reshape(128, NPIX).astype(NBF)
        vrc = vrows[c]
        l2in = np.empty((128, L2IN), NBF)
        l2in[:, 0:128] = pstack[b]
        l2in[:, 128:256] = woT
        for cc in range(4):
            base = 256 + 2048 * cc
            l2in[:, base:base + 1024] = vrc[:, 1024 * cc:1024 * (cc + 1)]
            l2in[:, base + 1024:base + 2048] = xr[:, 1024 * cc:1024 * (cc + 1)]
        maps2.append({"l2in": l2in})
    r2 = _run(_get("l2"), maps2, ("y_sl",))
    y = np.empty((B, C, H, W), f)
    for c in range(8):
        b, r0 = c // 4, 32 * (c % 4)
        y[b, :, r0:r0 + RPC, :] = r2[c]["y_sl"].reshape(C, RPC, W).astype(f)
    return y


def kernel_sim(**inputs):
    _CACHE["sim"] = True
    try:
        return kernel(**inputs)
    finally:
        _CACHE["sim"] = False
